# revision 24
# baseline (speedup 1.0000x reference)
"""CantorAttention Trainium2 kernel (v2).

Strategy
--------
8 cores = 2 (batch) x 4 (cluster-groups).  Queries are grouped host-side into
clusters of <=128 queries sharing a key-union of <=128 keys (the Cantor routes
have ~255 distinct rows -> ~52 clusters).  Each core owns NCC = ceil(NC/4)
clusters of one batch and computes ALL 16 heads plus the full out-projection
for its clusters' queries, so outputs are disjoint (no cross-core reduction).

All gathers/permutations/transposes of x happen on the HOST: each core
receives its clusters' query rows and key rows of x already gathered,
transposed (dim-major) and laid out in [128, 8x512] matmul-ready blocks.
On device:
  phase P (per 512-col block): QT = Wq^T x^T + bq, KT = Wk^T x^T + bk
      computed directly in transposed (dim-major) layout via big-N matmuls
      (stationary = W 128x128 chunk, moving = x^T block, N=512).
  phase A (per cluster): V = xkv @ Wv + bv (natural key-major layout);
      per 4-head chunk: scoresT_h = KT_h^T... i.e. matmul(lhsT=KT_h, rhs=QT_h)
      -> [128 keys, 128 q] PSUM; att = exp(SCALE*scoresT) (ACT) * maskT
      (multiplicative bf16 mask, DVE); AV: out_h = att_h^T @ V_h with an
      extra N=1 matmul against a ones-column reusing the same stationary att
      to get the softmax denominators; normalize with reciprocal+tensor_scalar;
      PE-transpose the [128q, 256] result to get OutT chunks; finally
      y = OutT^T @ W_out (8 accumulating N=512 matmuls) -> bf16 rows out.
Host scatters the per-cluster rows back to natural order and adds b_out.
"""

import math
import os
import sys

import ml_dtypes
import numpy as np

for _p in ("/opt/trn_rl_repo",):
    if os.path.isdir(_p) and _p not in sys.path:
        sys.path.insert(0, _p)

import concourse.bacc as bacc
import concourse.bass as bass
import concourse.mybir as mybir
import concourse.tile as tile
from concourse.bass_utils import run_bass_kernel_spmd
from concourse.masks import make_identity

B, S, DIM = 2, 4096, 1024
H, HD, KN = 16, 64, 64
SCALE = 1.0 / np.sqrt(HD).item()
QMAX = 128  # query slots per cluster
UMAX = 128  # max distinct keys per cluster
NCORES = 8
F32 = mybir.dt.float32
BF16 = mybir.dt.bfloat16
BF = ml_dtypes.bfloat16

# Heads of one 4-head attention unit share a PE row group (see _build); the
# resulting OutT chunk order is UNIT_HEADS flattened, so W_out rows are
# host-permuted to match.
UNIT_HEADS = [[0, 2, 4, 6], [1, 3, 5, 7], [8, 10, 12, 14], [9, 11, 13, 15]]
HEAD_PERM = [h for hu in UNIT_HEADS for h in hu]


# ---------------------------------------------------------------- host planning
def _plan_clusters(routes: np.ndarray):
    """Group queries by identical route rows, chain-order groups by key-set
    overlap, and greedily pack them into clusters of <=QMAX queries whose key
    union stays <=UMAX."""
    uniq, inv = np.unique(routes, axis=0, return_inverse=True)
    G = len(uniq)

    member = np.zeros((G, S), dtype=np.int8)
    for g in range(G):
        member[g, uniq[g]] = 1
    ov = member @ member.T
    order = [0]
    used = np.zeros(G, dtype=bool)
    used[0] = True
    for _ in range(G - 1):
        cand = np.where(~used)[0]
        nxt = int(cand[np.argmax(ov[order[-1], cand])])
        order.append(nxt)
        used[nxt] = True

    items = []  # (key_set, query_list)
    for g in order:
        qs = np.nonzero(inv == g)[0].tolist()
        ks = set(uniq[g].tolist())
        while len(qs) > QMAX:
            items.append((ks, qs[:QMAX]))
            qs = qs[QMAX:]
        if qs:
            items.append((ks, qs))

    clusters = []
    curq: list[int] = []
    curk: set[int] = set()
    for ks, qs in items:
        if len(curq) + len(qs) > QMAX or len(curk | ks) > UMAX:
            clusters.append((curq, sorted(curk)))
            curq, curk = [], set()
        curq = curq + qs
        curk = curk | ks
    if curq:
        clusters.append((curq, sorted(curk)))
    return clusters


class _Plan:
    def __init__(self, routes: np.ndarray):
        clusters = _plan_clusters(routes)
        NC = len(clusters)
        NCC = math.ceil(NC / 4)  # clusters per core (4 cluster-groups/batch)
        NBLK = math.ceil(NCC / 4)  # 512-query-col blocks per core
        NCpad = NCC * 4
        qidx = np.zeros((QMAX, NCpad), dtype=np.int32)
        kidx = np.zeros((QMAX, NCpad), dtype=np.int32)
        nvalid = np.zeros(NCpad, dtype=np.int32)
        # multiplicative transposed mask [keys, queries]
        maskT = np.zeros((NCpad, UMAX, QMAX), dtype=np.float32)
        for ci, (qs, ks) in enumerate(clusters):
            nv, nu = len(qs), len(ks)
            nvalid[ci] = nv
            qidx[:nv, ci] = qs
            kidx[:nu, ci] = ks
            ks_arr = np.asarray(ks, dtype=np.int32)
            hit = (routes[np.asarray(qs)][:, :, None] == ks_arr[None, None, :]).any(
                axis=1
            )  # [nv, nu]
            maskT[ci, :nu, :nv] = hit.T
            maskT[ci, 0, nv:] = 1.0  # keep pad-query denominators finite
        for ci in range(NC, NCpad):
            maskT[ci, 0, :] = 1.0
        self.NC, self.NCC, self.NBLK = NC, NCC, NBLK
        self.qidx, self.kidx, self.nvalid, self.maskT = qidx, kidx, nvalid, maskT


_PLAN_CACHE: dict = {}


def _get_plan(routes: np.ndarray) -> _Plan:
    key = routes.tobytes()
    if key not in _PLAN_CACHE:
        _PLAN_CACHE[key] = _Plan(routes)
    return _PLAN_CACHE[key]


def _prep_blocks(xb_bf: np.ndarray, rows: np.ndarray, nblk: int) -> np.ndarray:
    """Gather rows of x (bf16 [S, DIM]) and lay out as [NBLK, 128, 8*512]:
    block[p, k*512 + j] = x[rows[blk*512 + j], k*128 + p]."""
    r = np.zeros(nblk * 512, dtype=np.int64)
    r[: len(rows)] = rows
    a = xb_bf[r]  # [nblk*512, 1024]
    a = a.reshape(nblk, 512, 8, 128).transpose(0, 3, 2, 1)
    return np.ascontiguousarray(a).reshape(nblk, 128, 8 * 512)


def _make_in_maps(inputs):
    x = np.asarray(inputs["x"], dtype=np.float32)
    W_qkv = np.asarray(inputs["W_qkv"], dtype=np.float32)
    b_qkv = np.asarray(inputs["b_qkv"], dtype=np.float32)
    W_out = np.asarray(inputs["W_out"], dtype=np.float32)
    routes = np.asarray(inputs["routes"], dtype=np.int32)
    plan = _get_plan(routes)
    NCC, NBLK = plan.NCC, plan.NBLK

    x_bf = [np.ascontiguousarray(x[b]).astype(BF) for b in range(B)]
    wqkv = np.ascontiguousarray(W_qkv).astype(BF)
    bqkv = np.ascontiguousarray(b_qkv).reshape(1, 3 * DIM).astype(BF)
    # QT/KT biases as per-partition columns for the ACT-copy bias path
    bqcol = np.ascontiguousarray(
        b_qkv[: 2 * DIM].reshape(16, 128).T.astype(np.float32)
    )  # [128, 16]: cols 0-7 = Q chunks, 8-15 = K chunks
    wout = np.ascontiguousarray(
        W_out.reshape(H, HD, DIM)[HEAD_PERM].reshape(DIM, DIM)
    ).astype(BF)

    in_maps = []
    for c in range(NCORES):
        b, g = c // 4, c % 4
        lo, hi = g * NCC, (g + 1) * NCC
        rows_q = plan.qidx[:, lo:hi].T.reshape(-1)
        rows_k = plan.kidx[:, lo:hi].T.reshape(-1)
        xq_d = _prep_blocks(x_bf[b], rows_q, NBLK)
        xkv_d = _prep_blocks(x_bf[b], rows_k, NBLK)
        mask_d = np.zeros((NBLK, UMAX, 512), dtype=BF)
        for lc in range(NCC):
            blk, t = lc // 4, lc % 4
            mask_d[blk, :, t * 128 : (t + 1) * 128] = plan.maskT[lo + lc]
        in_maps.append(
            {
                "xq": xq_d,
                "xkv": xkv_d,
                "maskd": mask_d,
                "wqkv": wqkv,
                "bqkv": bqkv,
                "bqcol": bqcol,
                "wout": wout,
            }
        )
    return in_maps


# ---------------------------------------------------------------- device kernel
def _build(NCC: int, NBLK: int):
    nc = bacc.Bacc("TRN2", target_bir_lowering=False, debug=False, num_devices=NCORES)
    Exp = mybir.ActivationFunctionType.Exp
    Copy = mybir.ActivationFunctionType.Copy
    Ident = mybir.ActivationFunctionType.Identity
    mult = mybir.AluOpType.mult
    KV = int(os.environ.get("KV", "9"))  # bisect knob: 9 = full kernel

    xq = nc.dram_tensor("xq", [NBLK, 128, 8 * 512], BF16, kind="ExternalInput")
    xkv = nc.dram_tensor("xkv", [NBLK, 128, 8 * 512], BF16, kind="ExternalInput")
    maskd = nc.dram_tensor("maskd", [NBLK, UMAX, 512], BF16, kind="ExternalInput")
    wqkv = nc.dram_tensor("wqkv", [DIM, 3 * DIM], BF16, kind="ExternalInput")
    bqkv = nc.dram_tensor("bqkv", [1, 3 * DIM], BF16, kind="ExternalInput")
    bqcol = nc.dram_tensor("bqcol", [128, 16], F32, kind="ExternalInput")
    wout = nc.dram_tensor("wout", [DIM, DIM], BF16, kind="ExternalInput")
    yout = nc.dram_tensor("yout", [NCC * 128, DIM], BF16, kind="ExternalOutput")

    with tile.TileContext(nc) as tc:
        with (
            tc.tile_pool(name="const", bufs=1) as cp,
            tc.tile_pool(name="xin", bufs=2) as xp,
            tc.tile_pool(name="proj", bufs=2) as pp,
            tc.tile_pool(name="work", bufs=3) as wp,
            tc.tile_pool(name="ps", bufs=8, space="PSUM") as ps,
        ):
            # first block's activations load ahead of the (bigger) weights so
            # the first projection matmuls can start as early as possible
            blk0 = {}
            if KV >= 1:
                xq_t = xp.tile([128, 8 * 512], BF16, tag="xq_t")
                nc.sync.dma_start(xq_t[:], xq[0])
                xkv_t = xp.tile([128, 8 * 512], BF16, tag="xkv_t")
                nc.sync.dma_start(xkv_t[:], xkv[0])
                mask_t = xp.tile([128, 512], BF16, tag="mask_t")
                nc.sync.dma_start(mask_t[:], maskd[0])
                blk0 = {"xq_t": xq_t, "xkv_t": xkv_t, "mask_t": mask_t}

            idb = cp.tile([128, 128], BF16, tag="idb")
            make_identity(nc, idb[:])
            ones_b = cp.tile([1, 512], BF16, tag="ones_b")
            nc.gpsimd.memset(ones_b[:], 1.0)
            ones_c = cp.tile([128, 1], BF16, tag="ones_c")
            nc.gpsimd.memset(ones_c[:], 1.0)
            bq_sb = cp.tile([1, 3 * DIM], BF16, tag="bq")
            nc.sync.dma_start(bq_sb[:], bqkv[:])
            bqc_sb = cp.tile([128, 16], F32, tag="bqc")
            nc.sync.dma_start(bqc_sb[:], bqcol[:])
            w_sb = []
            for k in range(8):
                w = cp.tile([128, 3 * DIM], BF16, tag=f"w{k}")
                nc.sync.dma_start(w[:], wqkv[k * 128 : (k + 1) * 128, :])
                w_sb.append(w)
            wo_sb = []
            for k in range(8):
                w = cp.tile([128, DIM], BF16, tag=f"wo{k}")
                nc.sync.dma_start(w[:], wout[k * 128 : (k + 1) * 128, :])
                wo_sb.append(w)

            if KV == 0:
                z_t = wp.tile([128, DIM], BF16, tag="yb_t")
                nc.gpsimd.memset(z_t[:], 0.0)
                nc.sync.dma_start(yout[0:128, :], z_t[:])

            for blk in range(NBLK if KV >= 1 else 0):
                if blk == 0:
                    xq_t = blk0["xq_t"]
                    xkv_t = blk0["xkv_t"]
                    mask_t = blk0["mask_t"]
                else:
                    xq_t = xp.tile([128, 8 * 512], BF16, tag="xq_t")
                    nc.sync.dma_start(xq_t[:], xq[blk])
                    xkv_t = xp.tile([128, 8 * 512], BF16, tag="xkv_t")
                    nc.sync.dma_start(xkv_t[:], xkv[blk])
                    mask_t = xp.tile([128, 512], BF16, tag="mask_t")
                    nc.sync.dma_start(mask_t[:], maskd[blk])

                # ---- transposed projections QT/KT for this block ----
                qt_t = pp.tile([128, 8 * 512], BF16, tag="qt_t")
                kt_t = pp.tile([128, 8 * 512], BF16, tag="kt_t")
                for j in range(8):
                    pq = ps.tile([128, 512], F32, tag="ps")
                    for k in range(8):
                        nc.tensor.matmul(
                            pq[:],
                            lhsT=w_sb[k][:, j * 128 : (j + 1) * 128],
                            rhs=xq_t[:, k * 512 : (k + 1) * 512],
                            start=(k == 0),
                            stop=(k == 7),
                        )
                    nc.scalar.activation(
                        qt_t[:, j * 512 : (j + 1) * 512],
                        pq[:],
                        Ident,
                        bias=bqc_sb[:, j : j + 1],
                    )
                    pk = ps.tile([128, 512], F32, tag="ps")
                    for k in range(8):
                        nc.tensor.matmul(
                            pk[:],
                            lhsT=w_sb[k][:, DIM + j * 128 : DIM + (j + 1) * 128],
                            rhs=xkv_t[:, k * 512 : (k + 1) * 512],
                            start=(k == 0),
                            stop=(k == 7),
                        )
                    nc.scalar.activation(
                        kt_t[:, j * 512 : (j + 1) * 512],
                        pk[:],
                        Ident,
                        bias=bqc_sb[:, 8 + j : 9 + j],
                    )

                # ---- per-cluster attention + out-projection ----
                for lc in range(blk * 4, min((blk + 1) * 4, NCC) if KV >= 2 else 0):
                    off = (lc - blk * 4) * 128

                    # V = xkv_c @ Wv + bv  (natural [keys, dims] layout)
                    v_t = wp.tile([128, DIM], BF16, tag="v_t")
                    pv0 = ps.tile([128, 512], F32, tag="ps")
                    pv1 = ps.tile([128, 512], F32, tag="ps")
                    for k in range(8):
                        nc.tensor.matmul(
                            pv0[:],
                            lhsT=xkv_t[:, k * 512 + off : k * 512 + off + 128],
                            rhs=w_sb[k][:, 2 * DIM : 2 * DIM + 512],
                            start=(k == 0),
                            stop=False,
                        )
                        nc.tensor.matmul(
                            pv1[:],
                            lhsT=xkv_t[:, k * 512 + off : k * 512 + off + 128],
                            rhs=w_sb[k][:, 2 * DIM + 512 : 3 * DIM],
                            start=(k == 0),
                            stop=False,
                        )
                    nc.tensor.matmul(
                        pv0[:],
                        lhsT=ones_b[0:1, 0:128],
                        rhs=bq_sb[0:1, 2 * DIM : 2 * DIM + 512],
                        start=False,
                        stop=True,
                    )
                    nc.tensor.matmul(
                        pv1[:],
                        lhsT=ones_b[0:1, 0:128],
                        rhs=bq_sb[0:1, 2 * DIM + 512 : 3 * DIM],
                        start=False,
                        stop=True,
                    )
                    nc.scalar.activation(v_t[:, 0:512], pv0[:], Copy)
                    nc.scalar.activation(v_t[:, 512:1024], pv1[:], Copy)
                    if KV < 3:
                        continue

                    ot_t = wp.tile([128, DIM], BF16, tag="ot_t")
                    for u in range(4):  # 4-head units (same row group within a unit)
                        # Heads of one unit share r so their K=64 matmuls use the
                        # same PE row group and serialize in-array: concurrent
                        # different-row-group matmuls into one PSUM bank fault
                        # the device.  W_out rows are host-permuted to match.
                        hs = UNIT_HEADS[u]
                        pscore = ps.tile([128, 512], F32, tag="ps")
                        for hh in range(4):
                            h = hs[hh]
                            j, r = h // 2, (h % 2) * 64
                            nc.tensor.matmul(
                                pscore[:, hh * 128 : (hh + 1) * 128],
                                lhsT=kt_t[r : r + 64, j * 512 + off : j * 512 + off + 128],
                                rhs=qt_t[r : r + 64, j * 512 + off : j * 512 + off + 128],
                                start=True,
                                stop=True,
                            )
                        att_t = wp.tile([128, 512], BF16, tag="att_t")
                        nc.scalar.activation(att_t[:], pscore[:], Exp, scale=SCALE)
                        for hh in range(4):
                            nc.vector.tensor_tensor(
                                out=att_t[:, hh * 128 : (hh + 1) * 128],
                                in0=att_t[:, hh * 128 : (hh + 1) * 128],
                                in1=mask_t[:, off : off + 128],
                                op=mult,
                            )
                        if KV < 4:
                            continue
                        pav = ps.tile([128, 512], F32, tag="ps")
                        for hh in range(4):
                            h = hs[hh]
                            nc.tensor.matmul(
                                pav[:, hh * 64 : (hh + 1) * 64],
                                lhsT=att_t[:, hh * 128 : (hh + 1) * 128],
                                rhs=v_t[:, h * 64 : (h + 1) * 64],
                                start=True,
                                stop=True,
                            )
                            nc.tensor.matmul(
                                pav[:, 256 + hh : 257 + hh],
                                lhsT=att_t[:, hh * 128 : (hh + 1) * 128],
                                rhs=ones_c[:],
                                start=True,
                                stop=True,
                            )
                        rr = wp.tile([128, 4], F32, tag="rr")
                        nc.vector.reciprocal(rr[:], pav[:, 256:260])
                        on_t = wp.tile([128, 256], BF16, tag="on_t")
                        for hh in range(4):
                            nc.vector.tensor_scalar_mul(
                                on_t[:, hh * 64 : (hh + 1) * 64],
                                pav[:, hh * 64 : (hh + 1) * 64],
                                rr[:, hh : hh + 1],
                            )
                        if KV < 5:
                            continue
                        ptr = ps.tile([128, 256], BF16, tag="ps")
                        nc.tensor.transpose(ptr[:, 0:128], on_t[:, 0:128], idb[:])
                        nc.tensor.transpose(ptr[:, 128:256], on_t[:, 128:256], idb[:])
                        nc.vector.tensor_copy(ot_t[:, u * 256 : (u + 1) * 256], ptr[:])

                    if KV < 6:
                        continue
                    # y = OutT^T @ W_out
                    yb_t = wp.tile([128, DIM], BF16, tag="yb_t")
                    py0 = ps.tile([128, 512], F32, tag="ps")
                    py1 = ps.tile([128, 512], F32, tag="ps")
                    for i in range(8):
                        nc.tensor.matmul(
                            py0[:],
                            lhsT=ot_t[:, i * 128 : (i + 1) * 128],
                            rhs=wo_sb[i][:, 0:512],
                            start=(i == 0),
                            stop=(i == 7),
                        )
                        nc.tensor.matmul(
                            py1[:],
                            lhsT=ot_t[:, i * 128 : (i + 1) * 128],
                            rhs=wo_sb[i][:, 512:1024],
                            start=(i == 0),
                            stop=(i == 7),
                        )
                    nc.vector.tensor_copy(yb_t[:, 0:512], py0[:])
                    nc.vector.tensor_copy(yb_t[:, 512:1024], py1[:])
                    nc.sync.dma_start(yout[lc * 128 : (lc + 1) * 128, :], yb_t[:])
    nc.compile()
    return nc


_BUILD_CACHE: dict = {}


def _get_build(NCC: int, NBLK: int):
    key = (NCC, NBLK)
    if key not in _BUILD_CACHE:
        _BUILD_CACHE[key] = _build(NCC, NBLK)
    return _BUILD_CACHE[key]


def kernel(x, W_qkv, b_qkv, W_out, b_out, routes):
    b_out = np.asarray(b_out, dtype=np.float32)
    routes = np.asarray(routes, dtype=np.int32)
    plan = _get_plan(routes)
    nc = _get_build(plan.NCC, plan.NBLK)
    in_maps = _make_in_maps(
        {"x": x, "W_qkv": W_qkv, "b_qkv": b_qkv, "W_out": W_out, "routes": routes}
    )
    res = run_bass_kernel_spmd(nc, in_maps, list(range(NCORES)))

    NCC = plan.NCC
    y = np.empty((B, S, DIM), dtype=np.float32)
    for b in range(B):
        for g in range(4):
            yo = np.asarray(res.results[b * 4 + g]["yout"], dtype=np.float32)
            for lc in range(NCC):
                ci = g * NCC + lc
                if ci >= plan.NC:
                    continue
                nv = int(plan.nvalid[ci])
                qs = plan.qidx[:nv, ci]
                y[b][qs] = yo[lc * 128 : lc * 128 + nv]
    return y + b_out[None, None, :]


# revision 33
# speedup vs baseline: 1.0318x; 1.0318x over previous
"""CantorAttention Trainium2 kernel (v2).

Strategy
--------
8 cores = 2 (batch) x 4 (cluster-groups).  Queries are grouped host-side into
clusters of <=128 queries sharing a key-union of <=128 keys (the Cantor routes
have ~255 distinct rows -> ~52 clusters).  Each core owns NCC = ceil(NC/4)
clusters of one batch and computes ALL 16 heads plus the full out-projection
for its clusters' queries, so outputs are disjoint (no cross-core reduction).

All gathers/permutations/transposes of x happen on the HOST: each core
receives its clusters' query rows and key rows of x already gathered,
transposed (dim-major) and laid out in [128, 8x512] matmul-ready blocks.
On device:
  phase P (per 512-col block): QT = Wq^T x^T + bq, KT = Wk^T x^T + bk
      computed directly in transposed (dim-major) layout via big-N matmuls
      (stationary = W 128x128 chunk, moving = x^T block, N=512).
  phase A (per cluster): V = xkv @ Wv + bv (natural key-major layout);
      per 4-head chunk: scoresT_h = KT_h^T... i.e. matmul(lhsT=KT_h, rhs=QT_h)
      -> [128 keys, 128 q] PSUM; att = exp(SCALE*scoresT) (ACT) * maskT
      (multiplicative bf16 mask, DVE); AV: out_h = att_h^T @ V_h with an
      extra N=1 matmul against a ones-column reusing the same stationary att
      to get the softmax denominators; normalize with reciprocal+tensor_scalar;
      PE-transpose the [128q, 256] result to get OutT chunks; finally
      y = OutT^T @ W_out (8 accumulating N=512 matmuls) -> bf16 rows out.
Host scatters the per-cluster rows back to natural order and adds b_out.
"""

import math
import os
import sys

import ml_dtypes
import numpy as np

for _p in ("/opt/trn_rl_repo",):
    if os.path.isdir(_p) and _p not in sys.path:
        sys.path.insert(0, _p)

import concourse.bacc as bacc
import concourse.bass as bass
import concourse.mybir as mybir
import concourse.tile as tile
from concourse.bass_utils import run_bass_kernel_spmd
from concourse.masks import make_identity

B, S, DIM = 2, 4096, 1024
H, HD, KN = 16, 64, 64
SCALE = 1.0 / np.sqrt(HD).item()
QMAX = 128  # query slots per cluster
UMAX = 128  # max distinct keys per cluster
NCORES = 8
F32 = mybir.dt.float32
BF16 = mybir.dt.bfloat16
BF = ml_dtypes.bfloat16

# Heads of one 4-head attention unit share a PE row group (see _build); the
# resulting OutT chunk order is UNIT_HEADS flattened, so W_out rows are
# host-permuted to match.
UNIT_HEADS = [[0, 2, 4, 6], [1, 3, 5, 7], [8, 10, 12, 14], [9, 11, 13, 15]]
HEAD_PERM = [h for hu in UNIT_HEADS for h in hu]


# ---------------------------------------------------------------- host planning
def _plan_clusters(routes: np.ndarray):
    """Group queries by identical route rows, chain-order groups by key-set
    overlap, and greedily pack them into clusters of <=QMAX queries whose key
    union stays <=UMAX."""
    uniq, inv = np.unique(routes, axis=0, return_inverse=True)
    G = len(uniq)

    member = np.zeros((G, S), dtype=np.int8)
    for g in range(G):
        member[g, uniq[g]] = 1
    ov = member @ member.T
    order = [0]
    used = np.zeros(G, dtype=bool)
    used[0] = True
    for _ in range(G - 1):
        cand = np.where(~used)[0]
        nxt = int(cand[np.argmax(ov[order[-1], cand])])
        order.append(nxt)
        used[nxt] = True

    items = []  # (key_set, query_list)
    for g in order:
        qs = np.nonzero(inv == g)[0].tolist()
        ks = set(uniq[g].tolist())
        while len(qs) > QMAX:
            items.append((ks, qs[:QMAX]))
            qs = qs[QMAX:]
        if qs:
            items.append((ks, qs))

    clusters = []
    curq: list[int] = []
    curk: set[int] = set()
    for ks, qs in items:
        if len(curq) + len(qs) > QMAX or len(curk | ks) > UMAX:
            clusters.append((curq, sorted(curk)))
            curq, curk = [], set()
        curq = curq + qs
        curk = curk | ks
    if curq:
        clusters.append((curq, sorted(curk)))
    return clusters


class _Plan:
    def __init__(self, routes: np.ndarray):
        clusters = _plan_clusters(routes)
        NC = len(clusters)
        NCC = math.ceil(NC / 4)  # clusters per core (4 cluster-groups/batch)
        NBLK = math.ceil(NCC / 4)  # 512-query-col blocks per core
        NCpad = NCC * 4
        qidx = np.zeros((QMAX, NCpad), dtype=np.int32)
        kidx = np.zeros((QMAX, NCpad), dtype=np.int32)
        nvalid = np.zeros(NCpad, dtype=np.int32)
        # multiplicative transposed mask [keys, queries]
        maskT = np.zeros((NCpad, UMAX, QMAX), dtype=np.float32)
        for ci, (qs, ks) in enumerate(clusters):
            nv, nu = len(qs), len(ks)
            nvalid[ci] = nv
            qidx[:nv, ci] = qs
            kidx[:nu, ci] = ks
            ks_arr = np.asarray(ks, dtype=np.int32)
            hit = (routes[np.asarray(qs)][:, :, None] == ks_arr[None, None, :]).any(
                axis=1
            )  # [nv, nu]
            maskT[ci, :nu, :nv] = hit.T
            maskT[ci, 0, nv:] = 1.0  # keep pad-query denominators finite
        for ci in range(NC, NCpad):
            maskT[ci, 0, :] = 1.0
        self.NC, self.NCC, self.NBLK = NC, NCC, NBLK
        self.qidx, self.kidx, self.nvalid, self.maskT = qidx, kidx, nvalid, maskT


_PLAN_CACHE: dict = {}


def _get_plan(routes: np.ndarray) -> _Plan:
    key = routes.tobytes()
    if key not in _PLAN_CACHE:
        _PLAN_CACHE[key] = _Plan(routes)
    return _PLAN_CACHE[key]


def _prep_blocks(xb_bf: np.ndarray, rows: np.ndarray, nblk: int) -> np.ndarray:
    """Gather rows of x (bf16 [S, DIM]) and lay out as [NBLK, 128, 8*512]:
    block[p, k*512 + j] = x[rows[blk*512 + j], k*128 + p]."""
    r = np.zeros(nblk * 512, dtype=np.int64)
    r[: len(rows)] = rows
    a = xb_bf[r]  # [nblk*512, 1024]
    a = a.reshape(nblk, 512, 8, 128).transpose(0, 3, 2, 1)
    return np.ascontiguousarray(a).reshape(nblk, 128, 8 * 512)


def _make_in_maps(inputs):
    x = np.asarray(inputs["x"], dtype=np.float32)
    W_qkv = np.asarray(inputs["W_qkv"], dtype=np.float32)
    b_qkv = np.asarray(inputs["b_qkv"], dtype=np.float32)
    W_out = np.asarray(inputs["W_out"], dtype=np.float32)
    routes = np.asarray(inputs["routes"], dtype=np.int32)
    plan = _get_plan(routes)
    NCC, NBLK = plan.NCC, plan.NBLK

    x_bf = [np.ascontiguousarray(x[b]).astype(BF) for b in range(B)]
    wqkv = np.ascontiguousarray(W_qkv).astype(BF)
    # QT/KT biases as per-partition columns for the ACT-copy bias path
    bqcol = np.ascontiguousarray(
        b_qkv[: 2 * DIM].reshape(16, 128).T.astype(np.float32)
    )  # [128, 16]: cols 0-7 = Q chunks, 8-15 = K chunks
    wout = np.ascontiguousarray(
        W_out.reshape(H, HD, DIM)[HEAD_PERM].reshape(DIM, DIM)
    ).astype(BF)

    in_maps = []
    for c in range(NCORES):
        b, g = c // 4, c % 4
        lo, hi = g * NCC, (g + 1) * NCC
        rows_q = plan.qidx[:, lo:hi].T.reshape(-1)
        rows_k = plan.kidx[:, lo:hi].T.reshape(-1)
        xq_d = _prep_blocks(x_bf[b], rows_q, NBLK)
        xkv_d = _prep_blocks(x_bf[b], rows_k, NBLK)
        mask_d = np.zeros((NBLK, UMAX, 512), dtype=BF)
        for lc in range(NCC):
            blk, t = lc // 4, lc % 4
            mask_d[blk, :, t * 128 : (t + 1) * 128] = plan.maskT[lo + lc]
        in_maps.append(
            {
                "xq": xq_d,
                "xkv": xkv_d,
                "maskd": mask_d,
                "wqkv": wqkv,
                "bqcol": bqcol,
                "wout": wout,
            }
        )
    return in_maps


# ---------------------------------------------------------------- device kernel
def _build(NCC: int, NBLK: int):
    nc = bacc.Bacc("TRN2", target_bir_lowering=False, debug=False, num_devices=NCORES)
    Exp = mybir.ActivationFunctionType.Exp
    Copy = mybir.ActivationFunctionType.Copy
    Ident = mybir.ActivationFunctionType.Identity
    mult = mybir.AluOpType.mult
    KV = int(os.environ.get("KV", "9"))  # bisect knob: 9 = full kernel

    xq = nc.dram_tensor("xq", [NBLK, 128, 8 * 512], BF16, kind="ExternalInput")
    xkv = nc.dram_tensor("xkv", [NBLK, 128, 8 * 512], BF16, kind="ExternalInput")
    maskd = nc.dram_tensor("maskd", [NBLK, UMAX, 512], BF16, kind="ExternalInput")
    wqkv = nc.dram_tensor("wqkv", [DIM, 3 * DIM], BF16, kind="ExternalInput")
    bqcol = nc.dram_tensor("bqcol", [128, 16], F32, kind="ExternalInput")
    wout = nc.dram_tensor("wout", [DIM, DIM], BF16, kind="ExternalInput")
    yout = nc.dram_tensor("yout", [NCC * 128, DIM], BF16, kind="ExternalOutput")

    with tile.TileContext(nc) as tc:
        with (
            tc.tile_pool(name="const", bufs=1) as cp,
            tc.tile_pool(name="xin", bufs=2) as xp,
            tc.tile_pool(name="proj", bufs=2) as pp,
            tc.tile_pool(name="work", bufs=3) as wp,
            tc.tile_pool(name="ps", bufs=8, space="PSUM") as ps,
        ):
            # first block's activations load ahead of the (bigger) weights so
            # the first projection matmuls can start as early as possible
            def load_block(blk):
                xq_t = xp.tile([128, 8 * 512], BF16, tag="xq_t", name="xq_t")
                nc.sync.dma_start(xq_t[:], xq[blk])
                xkv_t = xp.tile([128, 8 * 512], BF16, tag="xkv_t", name="xkv_t")
                nc.sync.dma_start(xkv_t[:], xkv[blk])
                mask_t = xp.tile([128, 512], BF16, tag="mask_t", name="mask_t")
                nc.gpsimd.dma_start(mask_t[:], maskd[blk])
                return {"xq_t": xq_t, "xkv_t": xkv_t, "mask_t": mask_t}

            blk0 = {}
            if KV >= 1:
                blk0 = load_block(0)

            w_sb = []
            for k in range(8):
                w = cp.tile([128, 3 * DIM], BF16, tag=f"w{k}")
                nc.gpsimd.dma_start(w[:], wqkv[k * 128 : (k + 1) * 128, :])
                w_sb.append(w)
            idb = cp.tile([128, 128], BF16, tag="idb")
            make_identity(nc, idb[:])
            ones_c = cp.tile([128, 1], BF16, tag="ones_c")
            nc.gpsimd.memset(ones_c[:], 1.0)
            bqc_sb = cp.tile([128, 16], F32, tag="bqc")
            nc.gpsimd.dma_start(bqc_sb[:], bqcol[:])
            wo_sb = []
            for k in range(8):
                w = cp.tile([128, DIM], BF16, tag=f"wo{k}")
                nc.gpsimd.dma_start(w[:], wout[k * 128 : (k + 1) * 128, :])
                wo_sb.append(w)

            if KV == 0:
                z_t = wp.tile([128, DIM], BF16, tag="yb_t")
                nc.gpsimd.memset(z_t[:], 0.0)
                nc.sync.dma_start(yout[0:128, :], z_t[:])

            pending = None  # (ot_t, lc) whose out-projection is deferred

            def _emit_yproj(ot_t, lc):
                # y = OutT^T @ W_out
                yb_t = wp.tile([128, DIM], BF16, tag="yb_t", name="yb_t")
                py0 = ps.tile([128, 512], F32, tag="ps", name="py0")
                py1 = ps.tile([128, 512], F32, tag="ps", name="py1")
                for i in range(8):
                    nc.tensor.matmul(
                        py0[:],
                        lhsT=ot_t[:, i * 128 : (i + 1) * 128],
                        rhs=wo_sb[i][:, 0:512],
                        start=(i == 0),
                        stop=(i == 7),
                    )
                    nc.tensor.matmul(
                        py1[:],
                        lhsT=ot_t[:, i * 128 : (i + 1) * 128],
                        rhs=wo_sb[i][:, 512:1024],
                        start=(i == 0),
                        stop=(i == 7),
                    )
                nc.vector.tensor_copy(yb_t[:, 0:512], py0[:])
                nc.vector.tensor_copy(yb_t[:, 512:1024], py1[:])
                nc.sync.dma_start(yout[lc * 128 : (lc + 1) * 128, :], yb_t[:])

            for blk in range(NBLK if KV >= 1 else 0):
                d = blk0 if blk == 0 else load_block(blk)
                xq_t, xkv_t, mask_t = d["xq_t"], d["xkv_t"], d["mask_t"]
                bw = 128 * min(4, NCC - blk * 4)  # valid query-cols this block

                # ---- transposed projections QT/KT for this block ----
                qt_t = pp.tile([128, 8 * 512], BF16, tag="qt_t")
                kt_t = pp.tile([128, 8 * 512], BF16, tag="kt_t")
                for j in range(8):
                    pq = ps.tile([128, 512], F32, tag="ps")
                    for k in range(8):
                        nc.tensor.matmul(
                            pq[:, 0:bw],
                            lhsT=w_sb[k][:, j * 128 : (j + 1) * 128],
                            rhs=xq_t[:, k * 512 : k * 512 + bw],
                            start=(k == 0),
                            stop=(k == 7),
                        )
                    nc.scalar.activation(
                        qt_t[:, j * 512 : j * 512 + bw],
                        pq[:, 0:bw],
                        Ident,
                        bias=bqc_sb[:, j : j + 1],
                    )
                    pk = ps.tile([128, 512], F32, tag="ps")
                    for k in range(8):
                        nc.tensor.matmul(
                            pk[:, 0:bw],
                            lhsT=w_sb[k][:, DIM + j * 128 : DIM + (j + 1) * 128],
                            rhs=xkv_t[:, k * 512 : k * 512 + bw],
                            start=(k == 0),
                            stop=(k == 7),
                        )
                    nc.scalar.activation(
                        kt_t[:, j * 512 : j * 512 + bw],
                        pk[:, 0:bw],
                        Ident,
                        bias=bqc_sb[:, 8 + j : 9 + j],
                    )

                # ---- per-cluster attention + out-projection ----
                # Emission order is software-pipelined for the in-order PE:
                # scores(u+1) is issued before AV(u) so exp/mask latency hides,
                # all OutT transposes are batched after the last AV, and the
                # previous cluster's out-projection runs behind this cluster's
                # V projection.
                for lc in range(blk * 4, min((blk + 1) * 4, NCC) if KV >= 2 else 0):
                    off = (lc - blk * 4) * 128

                    # V = xkv_c @ Wv + bv  (natural [keys, dims] layout)
                    v_t = wp.tile([128, DIM], BF16, tag="v_t")
                    pv0 = ps.tile([128, 512], F32, tag="ps")
                    pv1 = ps.tile([128, 512], F32, tag="ps")
                    # V carries no bias: softmax rows sum to 1, so bv passes
                    # through attention additively and is folded into the host
                    # bias as bv @ W_out.
                    for k in range(8):
                        nc.tensor.matmul(
                            pv0[:],
                            lhsT=xkv_t[:, k * 512 + off : k * 512 + off + 128],
                            rhs=w_sb[k][:, 2 * DIM : 2 * DIM + 512],
                            start=(k == 0),
                            stop=(k == 7),
                        )
                        nc.tensor.matmul(
                            pv1[:],
                            lhsT=xkv_t[:, k * 512 + off : k * 512 + off + 128],
                            rhs=w_sb[k][:, 2 * DIM + 512 : 3 * DIM],
                            start=(k == 0),
                            stop=(k == 7),
                        )
                    if KV < 3:
                        nc.scalar.activation(v_t[:, 0:512], pv0[:], Copy)
                        nc.scalar.activation(v_t[:, 512:1024], pv1[:], Copy)
                        if pending is not None and KV >= 6:
                            _emit_yproj(*pending)
                            pending = None
                        continue

                    on_t = wp.tile([128, 1024], BF16, tag="on_t")

                    def emit_scores(u):
                        # Heads of one unit share r so their K=64 matmuls use
                        # the same PE row group and serialize in-array:
                        # concurrent different-row-group matmuls into one PSUM
                        # bank fault the device.  W_out rows are host-permuted
                        # to match the resulting OutT chunk order.
                        hs = UNIT_HEADS[u]
                        pscore = ps.tile([128, 512], F32, tag="ps", name="pscore")
                        for hh in range(4):
                            h = hs[hh]
                            j, r = h // 2, (h % 2) * 64
                            nc.tensor.matmul(
                                pscore[:, hh * 128 : (hh + 1) * 128],
                                lhsT=kt_t[
                                    r : r + 64, j * 512 + off : j * 512 + off + 128
                                ],
                                rhs=qt_t[
                                    r : r + 64, j * 512 + off : j * 512 + off + 128
                                ],
                                start=True,
                                stop=True,
                            )
                        att_t = wp.tile([128, 512], BF16, tag="att_t", name="att_t")
                        nc.scalar.activation(att_t[:], pscore[:], Exp, scale=SCALE)
                        av = att_t[:].rearrange("p (u n) -> p u n", u=4)
                        mv = (
                            mask_t[:, off : off + 128]
                            .rearrange("p (u n) -> p u n", u=1)
                            .broadcast_to([128, 4, 128])
                        )
                        nc.vector.tensor_tensor(out=av, in0=av, in1=mv, op=mult)
                        return att_t

                    def emit_av(u, att_t):
                        hs = UNIT_HEADS[u]
                        pav = ps.tile([128, 512], F32, tag="ps", name="pav")
                        for hh in range(4):
                            h = hs[hh]
                            nc.tensor.matmul(
                                pav[:, hh * 64 : (hh + 1) * 64],
                                lhsT=att_t[:, hh * 128 : (hh + 1) * 128],
                                rhs=v_t[:, h * 64 : (h + 1) * 64],
                                start=True,
                                stop=True,
                            )
                            nc.tensor.matmul(
                                pav[:, 256 + hh : 257 + hh],
                                lhsT=att_t[:, hh * 128 : (hh + 1) * 128],
                                rhs=ones_c[:],
                                start=True,
                                stop=True,
                            )
                        rr = wp.tile([128, 4], F32, tag="rr", name="rr")
                        nc.vector.reciprocal(rr[:], pav[:, 256:260])
                        ov = on_t[:, u * 256 : (u + 1) * 256].rearrange(
                            "p (g n) -> p g n", g=4
                        )
                        pv = pav[:, 0:256].rearrange("p (g n) -> p g n", g=4)
                        rv = rr[:].rearrange("p (g n) -> p g n", n=1).broadcast_to(
                            [128, 4, 64]
                        )
                        nc.vector.tensor_tensor(out=ov, in0=pv, in1=rv, op=mult)

                    atts = [emit_scores(0), emit_scores(1)]
                    # V copies queue on ACT behind exp(u0/u1); the previous
                    # cluster's out-projection fills PE while exp/mask drain.
                    nc.scalar.activation(v_t[:, 0:512], pv0[:], Copy)
                    nc.scalar.activation(v_t[:, 512:1024], pv1[:], Copy)
                    if pending is not None and KV >= 6:
                        _emit_yproj(*pending)
                        pending = None
                    if KV >= 4:
                        emit_av(0, atts[0])
                        atts.append(emit_scores(2))
                        emit_av(1, atts[1])
                        atts.append(emit_scores(3))
                        emit_av(2, atts[2])
                        emit_av(3, atts[3])
                    if KV < 5:
                        continue

                    ot_t = wp.tile([128, DIM], BF16, tag="ot_t")
                    for u in range(4):
                        ptr = ps.tile([128, 256], BF16, tag="ps", name="ptr")
                        nc.tensor.transpose(
                            ptr[:, 0:128], on_t[:, u * 256 : u * 256 + 128], idb[:]
                        )
                        nc.tensor.transpose(
                            ptr[:, 128:256],
                            on_t[:, u * 256 + 128 : (u + 1) * 256],
                            idb[:],
                        )
                        nc.vector.tensor_copy(ot_t[:, u * 256 : (u + 1) * 256], ptr[:])
                    pending = (ot_t, lc)

            if pending is not None and KV >= 6:
                _emit_yproj(*pending)
    nc.compile()
    return nc


_BUILD_CACHE: dict = {}


def _get_build(NCC: int, NBLK: int):
    key = (NCC, NBLK)
    if key not in _BUILD_CACHE:
        _BUILD_CACHE[key] = _build(NCC, NBLK)
    return _BUILD_CACHE[key]


def kernel(x, W_qkv, b_qkv, W_out, b_out, routes):
    b_out = np.asarray(b_out, dtype=np.float32)
    b_qkv = np.asarray(b_qkv, dtype=np.float32)
    W_out_f = np.asarray(W_out, dtype=np.float32)
    # V bias is dropped on device (softmax rows sum to 1): add bv @ W_out here
    b_out = b_out + b_qkv[2 * DIM :] @ W_out_f
    routes = np.asarray(routes, dtype=np.int32)
    plan = _get_plan(routes)
    nc = _get_build(plan.NCC, plan.NBLK)
    in_maps = _make_in_maps(
        {"x": x, "W_qkv": W_qkv, "b_qkv": b_qkv, "W_out": W_out, "routes": routes}
    )
    res = run_bass_kernel_spmd(nc, in_maps, list(range(NCORES)))

    NCC = plan.NCC
    y = np.empty((B, S, DIM), dtype=np.float32)
    for b in range(B):
        for g in range(4):
            yo = np.asarray(res.results[b * 4 + g]["yout"], dtype=np.float32)
            for lc in range(NCC):
                ci = g * NCC + lc
                if ci >= plan.NC:
                    continue
                nv = int(plan.nvalid[ci])
                qs = plan.qidx[:nv, ci]
                y[b][qs] = yo[lc * 128 : lc * 128 + nv]
    return y + b_out[None, None, :]


# revision 41
# speedup vs baseline: 1.1297x; 1.0949x over previous
"""CantorAttention Trainium2 kernel (v3).

Strategy
--------
8 cores = 2 (batch) x 4 (cluster-groups).  Queries are grouped host-side into
clusters of <=256 queries sharing a key-union of <=256 keys (the Cantor routes
have ~255 distinct rows -> ~20 clusters), each padded to a uniform 2x2 tile
shape (2 query tiles x 2 key tiles of 128).  Each core owns NCC = ceil(NC/4)
clusters of one batch and computes ALL 16 heads plus the full out-projection
for its clusters' queries, so outputs are disjoint (no cross-core reduction).

All gathers/permutations/transposes of x happen on the HOST: each core
receives its clusters' query rows and key rows of x already gathered,
transposed (dim-major) and laid out in [128, 8x512] matmul-ready blocks.
On device:
  phase P (per 512-col block): QT = Wq^T x^T + bq, KT = Wk^T x^T + bk
      computed directly in transposed (dim-major) layout via big-N matmuls
      (stationary = W 128x128 chunk, moving = x^T block, N=512); the bias is
      applied by the ACT engine during the PSUM->SBUF copy (per-partition
      bias AP).
  phase A (per cluster): V = xkv @ Wv per key tile (natural layout, no bias:
      softmax rows sum to 1 so bv is folded into the host-side output bias);
      per (4-head unit u, query tile s): scoresT = matmul(lhsT=KT_h, rhs=QT_h)
      per key tile -> [128 keys, 4x128 q] PSUM; att = exp(SCALE*scoresT) (ACT)
      * mask (multiplicative bf16, broadcast across the unit's heads, DVE);
      AV accumulates over both key tiles, with an extra N=1 matmul against a
      ones-column (same stationary att) accumulating the softmax denominators;
      normalize via reciprocal + one broadcast tensor_tensor; PE-transpose the
      [128q, 256] unit outputs into OutT chunks; y = OutT^T @ W_out.
Heads of one unit share a PE row group (concurrent different-row-group
matmuls into one PSUM bank fault the device), so W_out rows are host-permuted
to the resulting OutT chunk order.  Host scatters the per-cluster rows back
to natural order and adds b_out + bv @ W_out.
"""

import math
import os
import sys

import ml_dtypes
import numpy as np

for _p in ("/opt/trn_rl_repo",):
    if os.path.isdir(_p) and _p not in sys.path:
        sys.path.insert(0, _p)

import concourse.bacc as bacc
import concourse.bass as bass
import concourse.mybir as mybir
import concourse.tile as tile
from concourse.bass_utils import run_bass_kernel_spmd
from concourse.masks import make_identity

B, S, DIM = 2, 4096, 1024
H, HD, KN = 16, 64, 64
SCALE = 1.0 / np.sqrt(HD).item()
QMAX = 256  # query slots per cluster (2 tiles)
UMAX = 256  # max distinct keys per cluster (2 tiles)
NCORES = 8
F32 = mybir.dt.float32
BF16 = mybir.dt.bfloat16
BF = ml_dtypes.bfloat16

# Heads of one 4-head attention unit share a PE row group (see _build); the
# resulting OutT chunk order is UNIT_HEADS flattened, so W_out rows are
# host-permuted to match.
UNIT_HEADS = [[0, 2, 4, 6], [1, 3, 5, 7], [8, 10, 12, 14], [9, 11, 13, 15]]
HEAD_PERM = [h for hu in UNIT_HEADS for h in hu]


# ---------------------------------------------------------------- host planning
def _plan_clusters(routes: np.ndarray):
    """Group queries by identical route rows, chain-order groups by key-set
    overlap, and greedily pack them into clusters of <=QMAX queries whose key
    union stays <=UMAX."""
    uniq, inv = np.unique(routes, axis=0, return_inverse=True)
    G = len(uniq)

    member = np.zeros((G, S), dtype=np.int8)
    for g in range(G):
        member[g, uniq[g]] = 1
    ov = member @ member.T
    order = [0]
    used = np.zeros(G, dtype=bool)
    used[0] = True
    for _ in range(G - 1):
        cand = np.where(~used)[0]
        nxt = int(cand[np.argmax(ov[order[-1], cand])])
        order.append(nxt)
        used[nxt] = True

    items = []  # (key_set, query_list)
    for g in order:
        qs = np.nonzero(inv == g)[0].tolist()
        ks = set(uniq[g].tolist())
        while len(qs) > QMAX:
            items.append((ks, qs[:QMAX]))
            qs = qs[QMAX:]
        if qs:
            items.append((ks, qs))

    clusters = []
    curq: list[int] = []
    curk: set[int] = set()
    for ks, qs in items:
        if len(curq) + len(qs) > QMAX or len(curk | ks) > UMAX:
            clusters.append((curq, sorted(curk)))
            curq, curk = [], set()
        curq = curq + qs
        curk = curk | ks
    if curq:
        clusters.append((curq, sorted(curk)))
    return clusters


class _Plan:
    def __init__(self, routes: np.ndarray):
        clusters = _plan_clusters(routes)
        NC = len(clusters)
        NCC = math.ceil(NC / 4)  # clusters per core (4 cluster-groups/batch)
        NBLK = math.ceil(NCC / 2)  # 512-query-col blocks per core
        NCpad = NCC * 4
        qidx = np.zeros((QMAX, NCpad), dtype=np.int32)
        kidx = np.zeros((QMAX, NCpad), dtype=np.int32)
        nvalid = np.zeros(NCpad, dtype=np.int32)
        # multiplicative transposed mask, cols = t*256 + s*128 + r
        maskT = np.zeros((NCpad, 128, 512), dtype=np.float32)
        for ci, (qs, ks) in enumerate(clusters):
            nv, nu = len(qs), len(ks)
            nvalid[ci] = nv
            qidx[:nv, ci] = qs
            kidx[:nu, ci] = ks
            ks_arr = np.asarray(ks, dtype=np.int32)
            hit = (routes[np.asarray(qs)][:, :, None] == ks_arr[None, None, :]).any(
                axis=1
            )  # [nv, nu]
            hitT = np.zeros((UMAX, QMAX), dtype=np.float32)
            hitT[:nu, :nv] = hit.T
            hitT[0, nv:] = 1.0  # keep pad-query denominators finite (key tile 0)
            for t in range(2):
                for s in range(2):
                    maskT[ci, :, t * 256 + s * 128 : t * 256 + (s + 1) * 128] = hitT[
                        t * 128 : (t + 1) * 128, s * 128 : (s + 1) * 128
                    ]
        for ci in range(NC, NCpad):
            maskT[ci, 0, 0:256] = 1.0
        self.NC, self.NCC, self.NBLK = NC, NCC, NBLK
        self.qidx, self.kidx, self.nvalid, self.maskT = qidx, kidx, nvalid, maskT


_PLAN_CACHE: dict = {}


def _get_plan(routes: np.ndarray) -> _Plan:
    key = routes.tobytes()
    if key not in _PLAN_CACHE:
        _PLAN_CACHE[key] = _Plan(routes)
    return _PLAN_CACHE[key]


def _prep_blocks(xb_bf: np.ndarray, rows: np.ndarray, nblk: int) -> np.ndarray:
    """Gather rows of x (bf16 [S, DIM]) and lay out as [NBLK, 128, 8*512]:
    block[p, k*512 + j] = x[rows[blk*512 + j], k*128 + p]."""
    r = np.zeros(nblk * 512, dtype=np.int64)
    r[: len(rows)] = rows
    a = xb_bf[r]  # [nblk*512, 1024]
    a = a.reshape(nblk, 512, 8, 128).transpose(0, 3, 2, 1)
    return np.ascontiguousarray(a).reshape(nblk, 128, 8 * 512)


def _make_in_maps(inputs):
    x = np.asarray(inputs["x"], dtype=np.float32)
    W_qkv = np.asarray(inputs["W_qkv"], dtype=np.float32)
    b_qkv = np.asarray(inputs["b_qkv"], dtype=np.float32)
    W_out = np.asarray(inputs["W_out"], dtype=np.float32)
    routes = np.asarray(inputs["routes"], dtype=np.int32)
    plan = _get_plan(routes)
    NCC, NBLK = plan.NCC, plan.NBLK

    x_bf = [np.ascontiguousarray(x[b]).astype(BF) for b in range(B)]
    wqkv = np.ascontiguousarray(W_qkv).astype(BF)
    # QT/KT biases as per-partition columns for the ACT-copy bias path
    bqcol = np.ascontiguousarray(
        b_qkv[: 2 * DIM].reshape(16, 128).T.astype(np.float32)
    )  # [128, 16]: cols 0-7 = Q chunks, 8-15 = K chunks
    wout = np.ascontiguousarray(
        W_out.reshape(H, HD, DIM)[HEAD_PERM].reshape(DIM, DIM)
    ).astype(BF)

    in_maps = []
    for c in range(NCORES):
        b, g = c // 4, c % 4
        lo, hi = g * NCC, (g + 1) * NCC
        rows_q = plan.qidx[:, lo:hi].T.reshape(-1)
        rows_k = plan.kidx[:, lo:hi].T.reshape(-1)
        xq_d = _prep_blocks(x_bf[b], rows_q, NBLK)
        xkv_d = _prep_blocks(x_bf[b], rows_k, NBLK)
        mask_d = np.ascontiguousarray(plan.maskT[lo:hi]).astype(BF)
        in_maps.append(
            {
                "xq": xq_d,
                "xkv": xkv_d,
                "maskd": mask_d,
                "wqkv": wqkv,
                "bqcol": bqcol,
                "wout": wout,
            }
        )
    return in_maps


# ---------------------------------------------------------------- device kernel
def _build(NCC: int, NBLK: int):
    nc = bacc.Bacc("TRN2", target_bir_lowering=False, debug=False, num_devices=NCORES)
    Exp = mybir.ActivationFunctionType.Exp
    Copy = mybir.ActivationFunctionType.Copy
    Ident = mybir.ActivationFunctionType.Identity
    mult = mybir.AluOpType.mult
    KV = int(os.environ.get("KV", "9"))  # bisect knob: 9 = full kernel

    xq = nc.dram_tensor("xq", [NBLK, 128, 8 * 512], BF16, kind="ExternalInput")
    xkv = nc.dram_tensor("xkv", [NBLK, 128, 8 * 512], BF16, kind="ExternalInput")
    maskd = nc.dram_tensor("maskd", [NCC, 128, 512], BF16, kind="ExternalInput")
    wqkv = nc.dram_tensor("wqkv", [DIM, 3 * DIM], BF16, kind="ExternalInput")
    bqcol = nc.dram_tensor("bqcol", [128, 16], F32, kind="ExternalInput")
    wout = nc.dram_tensor("wout", [DIM, DIM], BF16, kind="ExternalInput")
    yout = nc.dram_tensor("yout", [NCC * 256, DIM], BF16, kind="ExternalOutput")

    NQT = NCC * 2  # total query tiles (128 cols each)

    with tile.TileContext(nc) as tc:
        with (
            tc.tile_pool(name="const", bufs=1) as cp,
            tc.tile_pool(name="xin", bufs=2) as xp,
            tc.tile_pool(name="proj", bufs=2) as pp,
            tc.tile_pool(name="work", bufs=3) as wp,
            tc.tile_pool(name="ps", bufs=5, space="PSUM") as ps,
            tc.tile_pool(name="psb", bufs=3, space="PSUM") as psb,
        ):
            # first block's activations load ahead of the (bigger) weights so
            # the first projection matmuls can start as early as possible
            def load_block(blk):
                xq_t = xp.tile([128, 8 * 512], BF16, tag="xq_t", name="xq_t")
                nc.sync.dma_start(xq_t[:], xq[blk])
                xkv_t = xp.tile([128, 8 * 512], BF16, tag="xkv_t", name="xkv_t")
                nc.sync.dma_start(xkv_t[:], xkv[blk])
                return {"xq_t": xq_t, "xkv_t": xkv_t}

            blk0 = {}
            if KV >= 1:
                blk0 = load_block(0)

            w_sb = []
            for k in range(8):
                w = cp.tile([128, 3 * DIM], BF16, tag=f"w{k}")
                w_sb.append(w)
            # Q parts first: the first projection matmuls gate only on these
            for part in range(3):
                for k in range(8):
                    nc.gpsimd.dma_start(
                        w_sb[k][:, part * DIM : (part + 1) * DIM],
                        wqkv[k * 128 : (k + 1) * 128, part * DIM : (part + 1) * DIM],
                    )
            idb = cp.tile([128, 128], BF16, tag="idb")
            make_identity(nc, idb[:])
            ones_c = cp.tile([128, 1], BF16, tag="ones_c")
            nc.gpsimd.memset(ones_c[:], 1.0)
            bqc_sb = cp.tile([128, 16], F32, tag="bqc")
            nc.gpsimd.dma_start(bqc_sb[:], bqcol[:])
            wo_sb = []
            for k in range(8):
                w = cp.tile([128, DIM], BF16, tag=f"wo{k}")
                nc.gpsimd.dma_start(w[:], wout[k * 128 : (k + 1) * 128, :])
                wo_sb.append(w)

            if KV == 0:
                z_t = wp.tile([128, DIM], BF16, tag="yb_t")
                nc.gpsimd.memset(z_t[:], 0.0)
                nc.sync.dma_start(yout[0:128, :], z_t[:])

            pending = None  # (ots, lc) whose out-projection is deferred

            def _emit_yproj(ots, lc):
                # y = OutT^T @ W_out  (per query tile)
                for s, ot_t in enumerate(ots):
                    yb_t = wp.tile([128, DIM], BF16, tag="yb_t", name="yb_t")
                    py0 = ps.tile([128, 512], F32, tag="ps", name="py0")
                    py1 = ps.tile([128, 512], F32, tag="ps", name="py1")
                    for i in range(8):
                        nc.tensor.matmul(
                            py0[:],
                            lhsT=ot_t[:, i * 128 : (i + 1) * 128],
                            rhs=wo_sb[i][:, 0:512],
                            start=(i == 0),
                            stop=(i == 7),
                        )
                        nc.tensor.matmul(
                            py1[:],
                            lhsT=ot_t[:, i * 128 : (i + 1) * 128],
                            rhs=wo_sb[i][:, 512:1024],
                            start=(i == 0),
                            stop=(i == 7),
                        )
                    nc.vector.tensor_copy(yb_t[:, 0:512], py0[:])
                    nc.vector.tensor_copy(yb_t[:, 512:1024], py1[:])
                    nc.sync.dma_start(
                        yout[lc * 256 + s * 128 : lc * 256 + (s + 1) * 128, :],
                        yb_t[:],
                    )

            for blk in range(NBLK if KV >= 1 else 0):
                d = blk0 if blk == 0 else load_block(blk)
                xq_t, xkv_t = d["xq_t"], d["xkv_t"]
                bw = 128 * min(4, NQT - blk * 4)  # valid query-cols this block

                # ---- transposed projections QT/KT for this block ----
                qt_t = pp.tile([128, 8 * 512], BF16, tag="qt_t")
                kt_t = pp.tile([128, 8 * 512], BF16, tag="kt_t")
                for j in range(8):
                    pq = ps.tile([128, 512], F32, tag="ps")
                    for k in range(8):
                        nc.tensor.matmul(
                            pq[:, 0:bw],
                            lhsT=w_sb[k][:, j * 128 : (j + 1) * 128],
                            rhs=xq_t[:, k * 512 : k * 512 + bw],
                            start=(k == 0),
                            stop=(k == 7),
                        )
                    nc.scalar.activation(
                        qt_t[:, j * 512 : j * 512 + bw],
                        pq[:, 0:bw],
                        Ident,
                        bias=bqc_sb[:, j : j + 1],
                    )
                    pk = ps.tile([128, 512], F32, tag="ps")
                    for k in range(8):
                        nc.tensor.matmul(
                            pk[:, 0:bw],
                            lhsT=w_sb[k][:, DIM + j * 128 : DIM + (j + 1) * 128],
                            rhs=xkv_t[:, k * 512 : k * 512 + bw],
                            start=(k == 0),
                            stop=(k == 7),
                        )
                    nc.scalar.activation(
                        kt_t[:, j * 512 : j * 512 + bw],
                        pk[:, 0:bw],
                        Ident,
                        bias=bqc_sb[:, 8 + j : 9 + j],
                    )

                # ---- per-cluster attention + out-projection ----
                # Emission order is software-pipelined for the in-order PE:
                # scores of subunit i+1 are issued before AV of subunit i so
                # exp/mask latency hides, OutT transposes are batched after the
                # last AV, and the previous cluster's out-projection runs
                # behind this cluster's V projection and first scores.
                for lc in range(blk * 2, min((blk + 1) * 2, NCC) if KV >= 2 else 0):
                    off = (lc - blk * 2) * 256  # cluster base col within block

                    mask_t = wp.tile([128, 512], BF16, tag="mask_t", name="mask_t")
                    nc.gpsimd.dma_start(mask_t[:], maskd[lc])

                    # V = xkv_c @ Wv per key tile (no bias: folded into host
                    # output bias since softmax rows sum to 1)
                    v_ts = []
                    for t in range(2):
                        v_t = wp.tile([128, DIM], BF16, tag=f"v_t{t}", name="v_t")
                        pv0 = ps.tile([128, 512], F32, tag="ps", name="pv0")
                        pv1 = ps.tile([128, 512], F32, tag="ps", name="pv1")
                        ko = off + t * 128
                        for k in range(8):
                            nc.tensor.matmul(
                                pv0[:],
                                lhsT=xkv_t[:, k * 512 + ko : k * 512 + ko + 128],
                                rhs=w_sb[k][:, 2 * DIM : 2 * DIM + 512],
                                start=(k == 0),
                                stop=(k == 7),
                            )
                            nc.tensor.matmul(
                                pv1[:],
                                lhsT=xkv_t[:, k * 512 + ko : k * 512 + ko + 128],
                                rhs=w_sb[k][:, 2 * DIM + 512 : 3 * DIM],
                                start=(k == 0),
                                stop=(k == 7),
                            )
                        nc.scalar.activation(v_t[:, 0:512], pv0[:], Copy)
                        nc.scalar.activation(v_t[:, 512:1024], pv1[:], Copy)
                        v_ts.append(v_t)

                    if KV < 3:
                        if pending is not None and KV >= 6:
                            _emit_yproj(*pending)
                            pending = None
                        continue

                    on_s = [
                        wp.tile([128, 1024], BF16, tag="on0", name="on0"),
                        wp.tile([128, 1024], BF16, tag="on1", name="on1"),
                    ]

                    def emit_scores(u, s):
                        # Heads of one unit share r: their K=64 matmuls use the
                        # same PE row group and serialize in-array (concurrent
                        # different-row-group matmuls into one PSUM bank fault
                        # the device).
                        hs = UNIT_HEADS[u]
                        atts = []
                        for t in range(2):
                            pscore = psb.tile([128, 512], F32, tag="psb", name="pscore")
                            for hh in range(4):
                                h = hs[hh]
                                j, r = h // 2, (h % 2) * 64
                                kc = j * 512 + off + t * 128
                                qc = j * 512 + off + s * 128
                                nc.tensor.matmul(
                                    pscore[:, hh * 128 : (hh + 1) * 128],
                                    lhsT=kt_t[r : r + 64, kc : kc + 128],
                                    rhs=qt_t[r : r + 64, qc : qc + 128],
                                    start=True,
                                    stop=True,
                                )
                            att_t = wp.tile([128, 512], BF16, tag="att_t", name="att_t")
                            nc.scalar.activation(att_t[:], pscore[:], Exp, scale=SCALE)
                            av = att_t[:].rearrange("p (u n) -> p u n", u=4)
                            mv = (
                                mask_t[:, t * 256 + s * 128 : t * 256 + (s + 1) * 128]
                                .rearrange("p (u n) -> p u n", u=1)
                                .broadcast_to([128, 4, 128])
                            )
                            nc.vector.tensor_tensor(out=av, in0=av, in1=mv, op=mult)
                            atts.append(att_t)
                        return atts

                    def emit_av(u, s, atts):
                        hs = UNIT_HEADS[u]
                        pav = psb.tile([128, 512], F32, tag="psb", name="pav")
                        for hh in range(4):
                            h = hs[hh]
                            for t in range(2):
                                nc.tensor.matmul(
                                    pav[:, hh * 64 : (hh + 1) * 64],
                                    lhsT=atts[t][:, hh * 128 : (hh + 1) * 128],
                                    rhs=v_ts[t][:, h * 64 : (h + 1) * 64],
                                    start=(t == 0),
                                    stop=(t == 1),
                                )
                            for t in range(2):
                                nc.tensor.matmul(
                                    pav[:, 256 + hh : 257 + hh],
                                    lhsT=atts[t][:, hh * 128 : (hh + 1) * 128],
                                    rhs=ones_c[:],
                                    start=(t == 0),
                                    stop=(t == 1),
                                )
                        rr = wp.tile([128, 4], F32, tag="rr", name="rr")
                        nc.vector.reciprocal(rr[:], pav[:, 256:260])
                        ov = on_s[s][:, u * 256 : (u + 1) * 256].rearrange(
                            "p (g n) -> p g n", g=4
                        )
                        pv = pav[:, 0:256].rearrange("p (g n) -> p g n", g=4)
                        rv = (
                            rr[:]
                            .rearrange("p (g n) -> p g n", n=1)
                            .broadcast_to([128, 4, 64])
                        )
                        nc.vector.tensor_tensor(out=ov, in0=pv, in1=rv, op=mult)

                    SU = [(u, s) for u in range(4) for s in range(2)]
                    attq = [(SU[0], emit_scores(*SU[0]))]
                    if KV >= 4:
                        attq.append((SU[1], emit_scores(*SU[1])))
                        if pending is not None and KV >= 6:
                            _emit_yproj(*pending)
                            pending = None
                        for i in range(2, 8):
                            (pu, psq), patts = attq.pop(0)
                            emit_av(pu, psq, patts)
                            attq.append((SU[i], emit_scores(*SU[i])))
                        for (pu, psq), patts in attq:
                            emit_av(pu, psq, patts)
                    if KV < 5:
                        continue

                    ots = []
                    for s in range(2):
                        ot_t = wp.tile([128, DIM], BF16, tag=f"ot{s}", name="ot_t")
                        for u in range(4):
                            ptr = psb.tile([128, 256], BF16, tag="psb", name="ptr")
                            nc.tensor.transpose(
                                ptr[:, 0:128],
                                on_s[s][:, u * 256 : u * 256 + 128],
                                idb[:],
                            )
                            nc.tensor.transpose(
                                ptr[:, 128:256],
                                on_s[s][:, u * 256 + 128 : (u + 1) * 256],
                                idb[:],
                            )
                            nc.vector.tensor_copy(
                                ot_t[:, u * 256 : (u + 1) * 256], ptr[:]
                            )
                        ots.append(ot_t)
                    pending = (ots, lc)

            if pending is not None and KV >= 6:
                _emit_yproj(*pending)
    nc.compile()
    return nc


_BUILD_CACHE: dict = {}


def _get_build(NCC: int, NBLK: int):
    key = (NCC, NBLK)
    if key not in _BUILD_CACHE:
        _BUILD_CACHE[key] = _build(NCC, NBLK)
    return _BUILD_CACHE[key]


def kernel(x, W_qkv, b_qkv, W_out, b_out, routes):
    b_out = np.asarray(b_out, dtype=np.float32)
    b_qkv = np.asarray(b_qkv, dtype=np.float32)
    W_out_f = np.asarray(W_out, dtype=np.float32)
    # V bias is dropped on device (softmax rows sum to 1): add bv @ W_out here
    b_out = b_out + b_qkv[2 * DIM :] @ W_out_f
    routes = np.asarray(routes, dtype=np.int32)
    plan = _get_plan(routes)
    nc = _get_build(plan.NCC, plan.NBLK)
    in_maps = _make_in_maps(
        {"x": x, "W_qkv": W_qkv, "b_qkv": b_qkv, "W_out": W_out, "routes": routes}
    )
    res = run_bass_kernel_spmd(nc, in_maps, list(range(NCORES)))

    NCC = plan.NCC
    y = np.empty((B, S, DIM), dtype=np.float32)
    for b in range(B):
        for g in range(4):
            yo = np.asarray(res.results[b * 4 + g]["yout"], dtype=np.float32)
            for lc in range(NCC):
                ci = g * NCC + lc
                if ci >= plan.NC:
                    continue
                nv = int(plan.nvalid[ci])
                qs = plan.qidx[:nv, ci]
                y[b][qs] = yo[lc * 256 : lc * 256 + nv]
    return y + b_out[None, None, :]
